# revision 9
# baseline (speedup 1.0000x reference)
"""KoLeo loss kernel for Trainium2 (8 NeuronCores, data-parallel rows).

reference semantics:
    x = l2_normalize(student_output)            # [B, D]
    dots = x @ x.T ; dots[i, i] = -1
    I = argmax(dots, 1)
    loss = -mean(log(||x - x[I] + eps|| + eps))

Since rows are unit-norm, ||x_i - x_j|| = sqrt(2 - 2 * dot(x_i, x_j)), so the
nearest-neighbor distance depends only on the max off-diagonal dot:
    loss = -0.5 * mean(ln(2 - 2 * max_j!=i dots[i, j]))
(the eps terms contribute ~1e-8 relative and are dropped).

Sharding: each core gets the full x^T, column-rotated so its own 1024 rows
come first, computes its [1024, 8192] slice of the gram matrix in bf16, and
reduces to a scalar partial sum of ln(2 - 2*maxdot). The rotation makes the
diagonal location core-invariant, so one SPMD program serves all 8 cores.
Host sums the 8 partials.

Per-core device schedule:
  1. cast-DMA x^T f32 -> bf16 SBUF (4 tiles of [128, 8192])
  2. xsq = x*x (ACT), column norms via ones-matmul (PE, broadcasts the sums
     across partitions), inv = exp(-0.5*ln(norm2)) (ACT; Rsqrt is banned)
  3. normalize x in place: x *= inv (DVE)
  4. gram slice: 8 row-tiles x 4 col-groups of [128, 2048] PSUM, K=4 matmuls
     per 512-slice; diagonal killed by one extra I.T @ (-64 shifted I) matmul
  5. row max per col-group (DVE reduce from PSUM), ln(2-2*max) (ACT),
     sum across rows (DVE + gpsimd partition reduce), scalar partial out
"""

import numpy as np
import ml_dtypes

import concourse.bacc as bacc
import concourse.tile as tile
from concourse import mybir, bass_isa
from concourse.bass_utils import run_bass_kernel_spmd

B, D = 8192, 512
N_CORES = 8
ROWS = B // N_CORES          # 1024 rows per core
P = 128                      # SBUF partitions
KT = D // P                  # 4 contraction tiles
M_TILES = ROWS // P          # 8 output row tiles
NT = 512                     # matmul moving free dim
CG = 2048                    # PSUM col-group width (4 banks)
N_CGROUPS = B // CG          # 4
DIAG_C = 64.0                # diagonal kill constant

F32 = mybir.dt.float32
BF16 = mybir.dt.bfloat16
AF = mybir.ActivationFunctionType
ALU = mybir.AluOpType

_CACHE: dict = {}


def _build():
    nc = bacc.Bacc(
        "TRN2", target_bir_lowering=False, debug=False, num_devices=N_CORES
    )
    xt = nc.declare_dram_parameter("xt", [D, B], F32, isOutput=False)
    ident = nc.declare_dram_parameter("ident", [P, P], BF16, isOutput=False)
    # ebig[p, 384 + p] = -DIAG_C, zero elsewhere; slicing [384-off : 896-off]
    # yields a [P, NT] tile with -DIAG_C at [p, off + p]
    ebig = nc.declare_dram_parameter("ebig", [P, NT + 3 * P], BF16, isOutput=False)
    partial = nc.declare_dram_parameter("partial", [1, 1], F32, isOutput=True)

    with tile.TileContext(nc) as tc:
        with (
            tc.tile_pool(name="big", bufs=1) as big,
            tc.tile_pool(name="work", bufs=2) as work,
            tc.tile_pool(name="small", bufs=2) as small,
        ):
            # --- load x^T (f32 DRAM -> bf16 SBUF cast during DMA) ---
            xbf = [
                big.tile([P, B], BF16, name=f"xbf{k}", tag=f"xbf{k}")
                for k in range(KT)
            ]
            for k in range(KT):
                nc.gpsimd.dma_start(xbf[k][:], xt[k * P : (k + 1) * P, :])

            ident_sb = big.tile([P, P], BF16, name="ident_sb", tag="ident_sb")
            ebig_sb = big.tile([P, NT + 3 * P], BF16, name="ebig_sb", tag="ebig_sb")
            ones_sb = big.tile([P, P], BF16, name="ones_sb", tag="ones_sb")
            nc.sync.dma_start(ident_sb[:], ident[:])
            nc.sync.dma_start(ebig_sb[:], ebig[:])
            nc.gpsimd.memset(ones_sb[:], 1.0)

            # --- squared entries (ACT; DVE is the bottleneck engine) ---
            xsq = [
                big.tile([P, B], BF16, name=f"xsq{k}", tag=f"xsq{k}")
                for k in range(KT)
            ]
            for k in range(KT):
                nc.scalar.activation(xsq[k][:], xbf[k][:], AF.Square)

            # --- column norms, broadcast across partitions via ones-matmul:
            # norm2[p, j] = sum_d x[d, j]^2 for every p; inv = 1/sqrt(norm2)
            # computed as exp(-0.5*ln(norm2)).
            inv = big.tile([P, B], BF16, name="inv", tag="inv")
            with tc.tile_pool(name="npsum", bufs=1, space="PSUM") as npsum:
                for g in range(N_CGROUPS):
                    nps = npsum.tile([P, CG], F32, name="nps", tag="nps")
                    for c in range(CG // NT):
                        col0 = g * CG + c * NT
                        for k in range(KT):
                            nc.tensor.matmul(
                                nps[:, c * NT : (c + 1) * NT],
                                ones_sb[:],
                                xsq[k][:, col0 : col0 + NT],
                                start=(k == 0),
                                stop=(k == KT - 1),
                            )
                    lntmp = work.tile([P, CG], F32, name="lntmp", tag="lntmp")
                    nc.scalar.activation(lntmp[:], nps[:], AF.Ln)
                    nc.scalar.activation(
                        inv[:, g * CG : (g + 1) * CG], lntmp[:], AF.Exp, scale=-0.5
                    )

            # --- normalize in place: x[d, j] *= inv[j] (inv is row-constant)
            for k in range(KT):
                nc.vector.tensor_mul(xbf[k][:], xbf[k][:], inv[:])

            # --- gram slice + row max ---
            two_sb = small.tile([P, 1], F32, name="two_sb", tag="two_sb")
            nc.gpsimd.memset(two_sb[:], 2.0)
            loglist = small.tile([P, M_TILES], F32, name="loglist", tag="loglist")
            with tc.tile_pool(name="gpsum", bufs=2, space="PSUM") as gpsum:
                for mi in range(M_TILES):
                    maxcols = small.tile(
                        [P, N_CGROUPS], F32, name="maxcols", tag="maxcols"
                    )
                    diag_c = (mi * P) // NT   # 512-slice index with the diagonal
                    for cg in range(N_CGROUPS):
                        g = gpsum.tile([P, CG], F32, name="g", tag="g")
                        for c in range(CG // NT):
                            nslice = cg * (CG // NT) + c  # global 512-slice index
                            col0 = cg * CG + c * NT
                            has_diag = nslice == diag_c
                            for k in range(KT):
                                nc.tensor.matmul(
                                    g[:, c * NT : (c + 1) * NT],
                                    xbf[k][:, mi * P : (mi + 1) * P],
                                    xbf[k][:, col0 : col0 + NT],
                                    start=(k == 0),
                                    stop=(k == KT - 1 and not has_diag),
                                )
                            if has_diag:
                                off = (mi * P) % NT
                                # adds -64 at diag position [p, off+p]
                                nc.tensor.matmul(
                                    g[:, c * NT : (c + 1) * NT],
                                    ident_sb[:],
                                    ebig_sb[:, 3 * P - off : 3 * P - off + NT],
                                    start=False,
                                    stop=True,
                                )
                        nc.vector.reduce_max(
                            maxcols[:, cg : cg + 1], g[:], axis=mybir.AxisListType.X
                        )
                    rowmax = small.tile([P, 1], F32, name="rowmax", tag="rowmax")
                    nc.vector.reduce_max(
                        rowmax[:], maxcols[:], axis=mybir.AxisListType.X
                    )
                    # ln(2 - 2*maxdot) = 2*ln(nearest-neighbor distance)
                    nc.scalar.activation(
                        loglist[:, mi : mi + 1],
                        rowmax[:],
                        AF.Ln,
                        bias=two_sb[:],
                        scale=-2.0,
                    )

            # --- final reduction to one scalar per core ---
            sumlog = small.tile([P, 1], F32, name="sumlog", tag="sumlog")
            nc.vector.reduce_sum(
                sumlog[:], loglist[:], axis=mybir.AxisListType.X
            )
            total = small.tile([P, 1], F32, name="total", tag="total")
            nc.gpsimd.partition_all_reduce(
                total[:], sumlog[:], P, bass_isa.ReduceOp.add
            )
            nc.sync.dma_start(partial[:], total[0:1, 0:1])

    nc.finalize()
    return nc


def _get_nc():
    if "nc" not in _CACHE:
        _CACHE["nc"] = _build()
    return _CACHE["nc"]


def _in_maps(x: np.ndarray) -> list[dict]:
    ident = np.eye(P, dtype=np.float32).astype(ml_dtypes.bfloat16)
    ebig = np.zeros((P, NT + 3 * P), dtype=np.float32)
    ebig[np.arange(P), 3 * P + np.arange(P)] = -DIAG_C
    ebig = ebig.astype(ml_dtypes.bfloat16)
    maps = []
    for m in range(N_CORES):
        xrot = np.concatenate([x[m * ROWS :], x[: m * ROWS]], axis=0)
        maps.append(
            {
                "xt": np.ascontiguousarray(xrot.T),
                "ident": ident,
                "ebig": ebig,
            }
        )
    return maps


def run_kernel(x: np.ndarray, **spmd_kwargs):
    """Returns (loss_scalar_f32, BassKernelResults)."""
    res = run_bass_kernel_spmd(
        _get_nc(), _in_maps(x), core_ids=list(range(N_CORES)), **spmd_kwargs
    )
    s = sum(float(res.results[m]["partial"][0, 0]) for m in range(N_CORES))
    loss = np.float32(-0.5 * s / B)
    return np.asarray(loss, dtype=np.float32), res


def kernel(student_output: np.ndarray) -> np.ndarray:
    x = np.ascontiguousarray(np.asarray(student_output, dtype=np.float32))
    loss, _ = run_kernel(x)
    return loss


# revision 13
# speedup vs baseline: 1.1821x; 1.1821x over previous
"""KoLeo loss kernel for Trainium2 (8 NeuronCores, data-parallel rows).

reference semantics:
    x = l2_normalize(student_output)            # [B, D]
    dots = x @ x.T ; dots[i, i] = -1
    I = argmax(dots, 1)
    loss = -mean(log(||x - x[I] + eps|| + eps))

Since rows are unit-norm, ||x_i - x_j|| = sqrt(2 - 2 * dot(x_i, x_j)), so the
nearest-neighbor distance depends only on the max off-diagonal dot:
    loss = -0.5 * mean(ln(2 - 2 * max_j!=i dots[i, j]))
(the eps terms contribute ~1e-8 relative and are dropped).

Sharding: each core gets the full x^T, column-rotated so its own 1024 rows
come first, computes its [1024, 8192] slice of the gram matrix in bf16, and
reduces to a scalar partial sum of ln(2 - 2*maxdot). The rotation makes the
diagonal location core-invariant, so one SPMD program serves all 8 cores.
Host sums the 8 partials.

Per-core device schedule:
  1. cast-DMA x^T f32 -> bf16 SBUF (4 tiles of [128, 8192])
  2. xsq = x*x (ACT), column norms via ones-matmul (PE, broadcasts the sums
     across partitions), inv = exp(-0.5*ln(norm2)) (ACT; Rsqrt is banned)
  3. normalize x in place: x *= inv (DVE)
  4. gram slice: 8 row-tiles x 4 col-groups of [128, 2048] PSUM, K=4 matmuls
     per 512-slice; diagonal killed by one extra I.T @ (-64 shifted I) matmul
  5. row max per col-group (DVE reduce from PSUM), ln(2-2*max) (ACT),
     sum across rows (DVE + gpsimd partition reduce), scalar partial out
"""

import numpy as np
import ml_dtypes

import concourse.bacc as bacc
import concourse.tile as tile
from concourse import mybir, bass_isa
from concourse.bass_utils import run_bass_kernel_spmd

B, D = 8192, 512
N_CORES = 8
ROWS = B // N_CORES          # 1024 rows per core
P = 128                      # SBUF partitions
KT = D // P                  # 4 contraction tiles
M_TILES = ROWS // P          # 8 output row tiles
NT = 512                     # matmul moving free dim
CG = 2048                    # column-group width for the load/norm pipeline
N_CGROUPS = B // CG          # 4
GW = 1024                    # gram PSUM tile width (2 banks)
NG = B // GW                 # 8 gram column groups
DIAG_C = 64.0                # diagonal kill constant

F32 = mybir.dt.float32
BF16 = mybir.dt.bfloat16
AF = mybir.ActivationFunctionType
ALU = mybir.AluOpType

_CACHE: dict = {}


def _build():
    nc = bacc.Bacc(
        "TRN2", target_bir_lowering=False, debug=False, num_devices=N_CORES
    )
    xt = nc.declare_dram_parameter("xt", [D, B], F32, isOutput=False)
    ident = nc.declare_dram_parameter("ident", [P, P], BF16, isOutput=False)
    # ebig[p, 384 + p] = -DIAG_C, zero elsewhere; slicing [384-off : 896-off]
    # yields a [P, NT] tile with -DIAG_C at [p, off + p]
    ebig = nc.declare_dram_parameter("ebig", [P, NT + 3 * P], BF16, isOutput=False)
    partial = nc.declare_dram_parameter("partial", [1, 1], F32, isOutput=True)

    with tile.TileContext(nc) as tc:
        with (
            tc.tile_pool(name="big", bufs=1) as big,
            tc.tile_pool(name="work", bufs=2) as work,
            tc.tile_pool(name="small", bufs=2) as small,
        ):
            ident_sb = big.tile([P, P], BF16, name="ident_sb", tag="ident_sb")
            ebig_sb = big.tile([P, NT + 3 * P], BF16, name="ebig_sb", tag="ebig_sb")
            ones_sb = big.tile([P, P], BF16, name="ones_sb", tag="ones_sb")
            nc.sync.dma_start(ident_sb[:], ident[:])
            nc.sync.dma_start(ebig_sb[:], ebig[:])
            nc.gpsimd.memset(ones_sb[:], 1.0)
            two_sb = small.tile([P, 1], F32, name="two_sb", tag="two_sb")
            nc.gpsimd.memset(two_sb[:], 2.0)

            # --- load x^T (f32 DRAM -> bf16 SBUF cast during DMA), in
            # column-group chunks so later stages can pipeline by cg ---
            xbf = [
                big.tile([P, B], BF16, name=f"xbf{k}", tag=f"xbf{k}")
                for k in range(KT)
            ]
            for cg in range(N_CGROUPS):
                cs = slice(cg * CG, (cg + 1) * CG)
                for k in range(KT):
                    nc.gpsimd.dma_start(
                        xbf[k][:, cs], xt[k * P : (k + 1) * P, cs]
                    )

            xsq = [
                big.tile([P, B], BF16, name=f"xsq{k}", tag=f"xsq{k}")
                for k in range(KT)
            ]
            inv = big.tile([P, B], BF16, name="inv", tag="inv")
            loglist = small.tile([P, M_TILES], F32, name="loglist", tag="loglist")
            # per (mi, 1024-wide column group) partial row-maxes
            maxall = small.tile([P, M_TILES * NG], F32, name="maxall", tag="maxall")

            with (
                tc.tile_pool(name="npsum", bufs=2, space="PSUM") as npsum,
                tc.tile_pool(name="gpsum", bufs=3, space="PSUM") as gpsum,
            ):
                for cg in range(N_CGROUPS):
                    cs = slice(cg * CG, (cg + 1) * CG)
                    # squared entries (ACT; DVE is the busier engine)
                    for k in range(KT):
                        nc.scalar.activation(xsq[k][:, cs], xbf[k][:, cs], AF.Square)
                    # column norms broadcast across partitions via ones-matmul:
                    # norm2[p, j] = sum_d x[d, j]^2; inv = exp(-0.5*ln(norm2))
                    for c in range(CG // NT):
                        col0 = cg * CG + c * NT
                        nps = npsum.tile([P, NT], F32, name="nps", tag="nps")
                        for k in range(KT):
                            nc.tensor.matmul(
                                nps[:],
                                ones_sb[:],
                                xsq[k][:, col0 : col0 + NT],
                                start=(k == 0),
                                stop=(k == KT - 1),
                            )
                        lntmp = work.tile([P, NT], F32, name="lntmp", tag="lntmp")
                        nc.scalar.activation(lntmp[:], nps[:], AF.Ln)
                        nc.scalar.activation(
                            inv[:, col0 : col0 + NT], lntmp[:], AF.Exp, scale=-0.5
                        )
                    # normalize in place: x[d, j] *= inv[j] (inv row-constant)
                    for k in range(KT):
                        nc.vector.tensor_mul(
                            xbf[k][:, cs], xbf[k][:, cs], inv[:, cs]
                        )
                    # gram slice rows x this column group, then row-max.
                    # G tiles are [128, 1024] (2 PSUM banks): 2 halves per cg.
                    for h in range(2):
                        for mi in range(M_TILES):
                            g = gpsum.tile([P, GW], F32, name="g", tag="g")
                            for c2 in range(GW // NT):
                                nslice = cg * (CG // NT) + h * (GW // NT) + c2
                                col0 = nslice * NT
                                po = c2 * NT
                                has_diag = nslice == (mi * P) // NT
                                for k in range(KT):
                                    nc.tensor.matmul(
                                        g[:, po : po + NT],
                                        xbf[k][:, mi * P : (mi + 1) * P],
                                        xbf[k][:, col0 : col0 + NT],
                                        start=(k == 0),
                                        stop=(k == KT - 1 and not has_diag),
                                    )
                                if has_diag:
                                    off = (mi * P) % NT
                                    # adds -64 at diag position [p, off+p]
                                    nc.tensor.matmul(
                                        g[:, po : po + NT],
                                        ident_sb[:],
                                        ebig_sb[:, 3 * P - off : 3 * P - off + NT],
                                        start=False,
                                        stop=True,
                                    )
                            ng = cg * 2 + h  # 1024-wide group index, 0..7
                            nc.vector.reduce_max(
                                maxall[:, mi * NG + ng : mi * NG + ng + 1],
                                g[:],
                                axis=mybir.AxisListType.X,
                            )

                for mi in range(M_TILES):
                    rowmax = small.tile([P, 1], F32, name="rowmax", tag="rowmax")
                    nc.vector.reduce_max(
                        rowmax[:],
                        maxall[:, mi * NG : (mi + 1) * NG],
                        axis=mybir.AxisListType.X,
                    )
                    # ln(2 - 2*maxdot) = 2*ln(nearest-neighbor distance)
                    nc.scalar.activation(
                        loglist[:, mi : mi + 1],
                        rowmax[:],
                        AF.Ln,
                        bias=two_sb[:],
                        scale=-2.0,
                    )

            # --- final reduction to one scalar per core ---
            sumlog = small.tile([P, 1], F32, name="sumlog", tag="sumlog")
            nc.vector.reduce_sum(
                sumlog[:], loglist[:], axis=mybir.AxisListType.X
            )
            total = small.tile([P, 1], F32, name="total", tag="total")
            nc.gpsimd.partition_all_reduce(
                total[:], sumlog[:], P, bass_isa.ReduceOp.add
            )
            nc.sync.dma_start(partial[:], total[0:1, 0:1])

    nc.finalize()
    return nc


def _get_nc():
    if "nc" not in _CACHE:
        _CACHE["nc"] = _build()
    return _CACHE["nc"]


def _in_maps(x: np.ndarray) -> list[dict]:
    ident = np.eye(P, dtype=np.float32).astype(ml_dtypes.bfloat16)
    ebig = np.zeros((P, NT + 3 * P), dtype=np.float32)
    ebig[np.arange(P), 3 * P + np.arange(P)] = -DIAG_C
    ebig = ebig.astype(ml_dtypes.bfloat16)
    maps = []
    for m in range(N_CORES):
        xrot = np.concatenate([x[m * ROWS :], x[: m * ROWS]], axis=0)
        maps.append(
            {
                "xt": np.ascontiguousarray(xrot.T),
                "ident": ident,
                "ebig": ebig,
            }
        )
    return maps


def run_kernel(x: np.ndarray, **spmd_kwargs):
    """Returns (loss_scalar_f32, BassKernelResults)."""
    res = run_bass_kernel_spmd(
        _get_nc(), _in_maps(x), core_ids=list(range(N_CORES)), **spmd_kwargs
    )
    s = sum(float(res.results[m]["partial"][0, 0]) for m in range(N_CORES))
    loss = np.float32(-0.5 * s / B)
    return np.asarray(loss, dtype=np.float32), res


def kernel(student_output: np.ndarray) -> np.ndarray:
    x = np.ascontiguousarray(np.asarray(student_output, dtype=np.float32))
    loss, _ = run_kernel(x)
    return loss


# revision 16
# speedup vs baseline: 1.2063x; 1.0205x over previous
"""KoLeo loss kernel for Trainium2 (8 NeuronCores, data-parallel rows).

reference semantics:
    x = l2_normalize(student_output)            # [B, D]
    dots = x @ x.T ; dots[i, i] = -1
    I = argmax(dots, 1)
    loss = -mean(log(||x - x[I] + eps|| + eps))

Since rows are unit-norm, ||x_i - x_j|| = sqrt(2 - 2 * dot(x_i, x_j)), so the
nearest-neighbor distance depends only on the max off-diagonal dot:
    loss = -0.5 * mean(ln(2 - 2 * max_j!=i dots[i, j]))
(the eps terms contribute ~1e-8 relative and are dropped).

Sharding: each core gets the full x^T, column-rotated so its own 1024 rows
come first, computes its [1024, 8192] slice of the gram matrix in bf16, and
reduces to a scalar partial sum of ln(2 - 2*maxdot). The rotation makes the
diagonal location core-invariant, so one SPMD program serves all 8 cores.
Host sums the 8 partials.

Per-core device schedule:
  1. cast-DMA x^T f32 -> bf16 SBUF (4 tiles of [128, 8192])
  2. xsq = x*x (ACT), column norms via ones-matmul (PE, broadcasts the sums
     across partitions), inv = exp(-0.5*ln(norm2)) (ACT; Rsqrt is banned)
  3. normalize x in place: x *= inv (DVE)
  4. gram slice: 8 row-tiles x 4 col-groups of [128, 2048] PSUM, K=4 matmuls
     per 512-slice; diagonal killed by one extra I.T @ (-64 shifted I) matmul
  5. row max per col-group (DVE reduce from PSUM), ln(2-2*max) (ACT),
     sum across rows (DVE + gpsimd partition reduce), scalar partial out
"""

import numpy as np
import ml_dtypes

import concourse.bacc as bacc
import concourse.tile as tile
from concourse import mybir, bass_isa
from concourse.bass_utils import run_bass_kernel_spmd

B, D = 8192, 512
N_CORES = 8
ROWS = B // N_CORES          # 1024 rows per core
P = 128                      # SBUF partitions
KT = D // P                  # 4 contraction tiles
M_TILES = ROWS // P          # 8 output row tiles
NT = 512                     # matmul moving free dim
CG = 2048                    # column-group width for the load/norm pipeline
N_CGROUPS = B // CG          # 4
GW = 1024                    # gram PSUM tile width (2 banks)
NG = B // GW                 # 8 gram column groups
DIAG_C = 64.0                # diagonal kill constant

F32 = mybir.dt.float32
BF16 = mybir.dt.bfloat16
AF = mybir.ActivationFunctionType
ALU = mybir.AluOpType

_CACHE: dict = {}


def _build():
    nc = bacc.Bacc(
        "TRN2", target_bir_lowering=False, debug=False, num_devices=N_CORES
    )
    xt = nc.declare_dram_parameter("xt", [D, B], F32, isOutput=False)
    ident = nc.declare_dram_parameter("ident", [P, P], BF16, isOutput=False)
    # ebig[p, 384 + p] = -DIAG_C, zero elsewhere; slicing [384-off : 896-off]
    # yields a [P, NT] tile with -DIAG_C at [p, off + p]
    ebig = nc.declare_dram_parameter("ebig", [P, NT + 3 * P], BF16, isOutput=False)
    partial = nc.declare_dram_parameter("partial", [1, 1], F32, isOutput=True)

    with tile.TileContext(nc) as tc:
        with (
            tc.tile_pool(name="big", bufs=1) as big,
            tc.tile_pool(name="work", bufs=2) as work,
            tc.tile_pool(name="small", bufs=2) as small,
        ):
            ident_sb = big.tile([P, P], BF16, name="ident_sb", tag="ident_sb")
            ebig_sb = big.tile([P, NT + 3 * P], BF16, name="ebig_sb", tag="ebig_sb")
            ones_sb = big.tile([P, P], BF16, name="ones_sb", tag="ones_sb")
            nc.sync.dma_start(ident_sb[:], ident[:])
            nc.sync.dma_start(ebig_sb[:], ebig[:])
            nc.gpsimd.memset(ones_sb[:], 1.0)
            two_sb = small.tile([P, 1], F32, name="two_sb", tag="two_sb")
            nc.gpsimd.memset(two_sb[:], 2.0)

            # --- load x^T (f32 DRAM -> bf16 SBUF cast during DMA), in
            # column-group chunks so later stages can pipeline by cg ---
            xbf = [
                big.tile([P, B], BF16, name=f"xbf{k}", tag=f"xbf{k}")
                for k in range(KT)
            ]
            for cg in range(N_CGROUPS):
                cs = slice(cg * CG, (cg + 1) * CG)
                for k in range(KT):
                    nc.gpsimd.dma_start(
                        xbf[k][:, cs], xt[k * P : (k + 1) * P, cs]
                    )

            xsq = [
                big.tile([P, B], BF16, name=f"xsq{k}", tag=f"xsq{k}")
                for k in range(KT)
            ]
            inv = big.tile([P, B], BF16, name="inv", tag="inv")
            loglist = small.tile([P, M_TILES], F32, name="loglist", tag="loglist")
            # per (mi, 1024-wide column group) partial row-maxes
            maxall = small.tile([P, M_TILES * NG], F32, name="maxall", tag="maxall")

            with (
                tc.tile_pool(name="npsum", bufs=2, space="PSUM") as npsum,
                tc.tile_pool(name="gpsum", bufs=3, space="PSUM") as gpsum,
            ):
                for cg in range(N_CGROUPS):
                    cs = slice(cg * CG, (cg + 1) * CG)
                    # squared entries (DVE: keeping ACT to Ln/Exp only avoids
                    # activation-table thrash between sqrt_* and ln/exp sets)
                    for k in range(KT):
                        nc.vector.tensor_mul(xsq[k][:, cs], xbf[k][:, cs], xbf[k][:, cs])
                    # column norms broadcast across partitions via ones-matmul:
                    # norm2[p, j] = sum_d x[d, j]^2; inv = exp(-0.5*ln(norm2))
                    for c in range(CG // NT):
                        col0 = cg * CG + c * NT
                        nps = npsum.tile([P, NT], F32, name="nps", tag="nps")
                        for k in range(KT):
                            nc.tensor.matmul(
                                nps[:],
                                ones_sb[:],
                                xsq[k][:, col0 : col0 + NT],
                                start=(k == 0),
                                stop=(k == KT - 1),
                            )
                        lntmp = work.tile([P, NT], F32, name="lntmp", tag="lntmp")
                        nc.scalar.activation(lntmp[:], nps[:], AF.Ln)
                        nc.scalar.activation(
                            inv[:, col0 : col0 + NT], lntmp[:], AF.Exp, scale=-0.5
                        )
                    # normalize in place: x[d, j] *= inv[j] (inv row-constant).
                    # cg0 gates the first matmuls (lhsT lives in cg0 columns),
                    # so it runs on the faster DVE; later cgs overlap with
                    # matmuls of the previous cg and go to the idle GpSimd.
                    mul_eng = nc.vector if cg == 0 else nc.gpsimd
                    for k in range(KT):
                        mul_eng.tensor_mul(
                            xbf[k][:, cs], xbf[k][:, cs], inv[:, cs]
                        )
                    # gram slice rows x this column group, then row-max.
                    # G tiles are [128, 1024] (2 PSUM banks): 2 halves per cg.
                    for h in range(2):
                        for mi in range(M_TILES):
                            g = gpsum.tile([P, GW], F32, name="g", tag="g")
                            base = cg * (CG // NT) + h * (GW // NT)
                            diag_c2 = (mi * P) // NT - base  # -1ish if not here
                            # k outer: one LDWEIGHTS serves both 512-slices
                            for k in range(KT):
                                for c2 in range(GW // NT):
                                    nc.tensor.matmul(
                                        g[:, c2 * NT : (c2 + 1) * NT],
                                        xbf[k][:, mi * P : (mi + 1) * P],
                                        xbf[k][:, (base + c2) * NT : (base + c2 + 1) * NT],
                                        start=(k == 0),
                                        stop=(k == KT - 1 and c2 != diag_c2),
                                    )
                            if 0 <= diag_c2 < GW // NT:
                                off = (mi * P) % NT
                                # adds -64 at diag position [p, off+p]
                                nc.tensor.matmul(
                                    g[:, diag_c2 * NT : (diag_c2 + 1) * NT],
                                    ident_sb[:],
                                    ebig_sb[:, 3 * P - off : 3 * P - off + NT],
                                    start=False,
                                    stop=True,
                                )
                            ng = cg * 2 + h  # 1024-wide group index, 0..7
                            nc.vector.reduce_max(
                                maxall[:, mi * NG + ng : mi * NG + ng + 1],
                                g[:],
                                axis=mybir.AxisListType.X,
                            )

                for mi in range(M_TILES):
                    rowmax = small.tile([P, 1], F32, name="rowmax", tag="rowmax")
                    nc.vector.reduce_max(
                        rowmax[:],
                        maxall[:, mi * NG : (mi + 1) * NG],
                        axis=mybir.AxisListType.X,
                    )
                    # ln(2 - 2*maxdot) = 2*ln(nearest-neighbor distance)
                    nc.scalar.activation(
                        loglist[:, mi : mi + 1],
                        rowmax[:],
                        AF.Ln,
                        bias=two_sb[:],
                        scale=-2.0,
                    )

            # --- final reduction to one scalar per core ---
            sumlog = small.tile([P, 1], F32, name="sumlog", tag="sumlog")
            nc.vector.reduce_sum(
                sumlog[:], loglist[:], axis=mybir.AxisListType.X
            )
            total = small.tile([P, 1], F32, name="total", tag="total")
            nc.gpsimd.partition_all_reduce(
                total[:], sumlog[:], P, bass_isa.ReduceOp.add
            )
            nc.sync.dma_start(partial[:], total[0:1, 0:1])

    nc.finalize()
    return nc


def _get_nc():
    if "nc" not in _CACHE:
        _CACHE["nc"] = _build()
    return _CACHE["nc"]


def _in_maps(x: np.ndarray) -> list[dict]:
    ident = np.eye(P, dtype=np.float32).astype(ml_dtypes.bfloat16)
    ebig = np.zeros((P, NT + 3 * P), dtype=np.float32)
    ebig[np.arange(P), 3 * P + np.arange(P)] = -DIAG_C
    ebig = ebig.astype(ml_dtypes.bfloat16)
    maps = []
    for m in range(N_CORES):
        xrot = np.concatenate([x[m * ROWS :], x[: m * ROWS]], axis=0)
        maps.append(
            {
                "xt": np.ascontiguousarray(xrot.T),
                "ident": ident,
                "ebig": ebig,
            }
        )
    return maps


def run_kernel(x: np.ndarray, **spmd_kwargs):
    """Returns (loss_scalar_f32, BassKernelResults)."""
    res = run_bass_kernel_spmd(
        _get_nc(), _in_maps(x), core_ids=list(range(N_CORES)), **spmd_kwargs
    )
    s = sum(float(res.results[m]["partial"][0, 0]) for m in range(N_CORES))
    loss = np.float32(-0.5 * s / B)
    return np.asarray(loss, dtype=np.float32), res


def kernel(student_output: np.ndarray) -> np.ndarray:
    x = np.ascontiguousarray(np.asarray(student_output, dtype=np.float32))
    loss, _ = run_kernel(x)
    return loss
